# revision 27
# baseline (speedup 1.0000x reference)
"""Trainium2 Bass kernel for nn_Attention_Module (sparse_attention).

Computation per batch b (x_b: [C=256, T=4096] fp32):
    energy = x_b @ x_b^T                      # (256, 256), K=4096
    attn   = softmax(rowmax(energy) - energy) # == exp(mu - e)/Z, mu = rowmin
    out    = gamma * (attn @ x_b) + x_b

Strategy (8 cores, pure data-parallel, 4 batches/core):
  - Host pre-swizzles both x layouts so every DMA transfer is a fat
    contiguous run (16-32 KB per partition): fine-grained descriptors
    previously made HWDGE dispatch the bottleneck.
  - xt (t-on-partition, fp32) feeds the energy matmul as float32r;
    xn (c-on-partition, fp16) feeds matmul2 and the residual.
  - Attention is built as A''' = gamma*P/Z + I in natural [i, j] layout
    (gamma, 1/Z and the +x residual all folded in), transposed by ONE
    small xbar DMA per batch to At[j', 2m+jb, i'].  The xbar runs on the
    ACT HWDGE ring with NO other DMA on that ring (concurrent same-ring
    DMA dispatch corrupts xbar output).  matmul2 (A'''^T as fp16
    weights) then produces the FINAL output directly; the epilogue is a
    pure fp32->fp16 PSUM copy on the vector engine.
  - matmul2 is ordered in same-weight runs of 4 so weight reloads are
    elided.  Exactly 7 HWDGE DMAs per iteration (2 loads + 1 xbar + 4
    stores) so the 8 round-robin completion lanes never stall on a
    same-iteration predecessor.
  - Output stored fp16 (tolerance 2e-2); host upcasts.
"""

import numpy as np

B, C, T = 32, 256, 4096
NCORES = 8
NB = B // NCORES  # batches per core
P = 128
KT = T // P  # 32 k-tiles for the energy matmul
TC = T // 512  # 8 t-chunks for matmul2

_CACHE = {}


def _build_nc(variant=None):
    variant = variant or {}
    from contextlib import ExitStack

    import concourse.bacc as bacc
    import concourse.bass as bass
    import concourse.tile as tile
    from concourse import mybir

    f32 = mybir.dt.float32
    f32r = mybir.dt.float32r
    f16 = mybir.dt.float16
    ts = bass.ts

    nc = bacc.Bacc(
        "TRN2",
        target_bir_lowering=False,
        debug=False,
        enable_asserts=False,
        num_devices=NCORES,
    )

    # host-swizzled: xt[b, p, k*C + c] = x[b, c, k*128 + p]
    xt_h = nc.dram_tensor("xt", [NB, P, KT * C], f32r, kind="ExternalInput")
    # xn[b, p, m*T + t] = x[b, m*128 + p, t]  (fp16)
    xn_h = nc.dram_tensor("xn", [NB, P, 2 * T], f16, kind="ExternalInput")
    # aux: [gamma, pad, pad, pad, identity-row(128) fp32]
    aux_h = nc.dram_tensor("aux", [P, 132], f32, kind="ExternalInput")
    o_h = nc.dram_tensor("o", [NB, P, 2 * T], f16, kind="ExternalOutput")

    with tile.TileContext(nc) as tc:
        with ExitStack() as ctx:
            singles = ctx.enter_context(tc.tile_pool(name="singles", bufs=1))
            xt_pool = ctx.enter_context(tc.tile_pool(name="xt", bufs=2))
            xq_pool = ctx.enter_context(tc.tile_pool(name="xq", bufs=1))
            xn_pool = ctx.enter_context(tc.tile_pool(name="xn", bufs=3))
            out_pool = ctx.enter_context(tc.tile_pool(name="out", bufs=3))
            att_pool = ctx.enter_context(tc.tile_pool(name="att", bufs=2))
            sm_pool = ctx.enter_context(tc.tile_pool(name="sm", bufs=2))
            psum_e = ctx.enter_context(
                tc.tile_pool(name="psum_e", bufs=2, space="PSUM")
            )
            psum_o = ctx.enter_context(
                tc.tile_pool(name="psum_o", bufs=2, space="PSUM")
            )
            psum_t = ctx.enter_context(
                tc.tile_pool(name="psum_t", bufs=2, space="PSUM")
            )

            xt_ap = xt_h.ap()
            xn_ap = xn_h.ap()
            o_ap = o_h.ap()

            # aux on the SYNC ring; the ACT ring is xbar-exclusive
            aux = singles.tile([P, 132], f32)
            nc.sync.dma_start(aux[:], aux_h.ap())
            gv = aux[:, 0:1]
            ident16 = singles.tile([P, P], f16)
            nc.gpsimd.tensor_copy(ident16[:], aux[:, 4:132])

            # b0 loads split finer so matmul1 starts ASAP
            B0_SPANS = [(0, 4), (4, 4), (8, 8), (16, 8), (24, 8)]

            def issue_loads(b):
                if b == 0:
                    tls = []
                    for i, (k0, kn) in enumerate(B0_SPANS):
                        t_ = xq_pool.tile(
                            [P, kn, C], f32r, tag=f"xq{i}", name=f"xq{i}"
                        )
                        nc.sync.dma_start(
                            t_[:], xt_ap[b][:, k0 * C : (k0 + kn) * C]
                        )
                        tls.append((k0, kn, t_))
                else:
                    xta = xt_pool.tile([P, KT, C], f32r, tag="xta", name="xta")
                    nc.sync.dma_start(xta[:], xt_ap[b])
                    tls = [(0, KT, xta)]
                xn = xn_pool.tile([P, 2, T], f16, tag="xn", name="xn")
                nc.sync.dma_start(xn[:], xn_ap[b])
                return tls, xn

            def src_at(tls, k):
                for k0, kn, t_ in tls:
                    if k0 <= k < k0 + kn:
                        return t_, k - k0
                raise AssertionError

            tiles = {0: issue_loads(0)}
            pending = None  # (b, At, xn) awaiting matmul2

            for b in range(NB):
                tls, xn = tiles.pop(b)
                if b + 1 < NB:
                    tiles[b + 1] = issue_loads(b + 1)

                # ---- matmul1: energy blocks (both m in ONE psum bank) ----
                P2 = sm_pool.tile([P, 2, C], f16, tag="P2", name="P2")
                pe = psum_e.tile([P, 2, C], mybir.dt.float32, name="pe")
                for m in range(2):
                    for k in range(KT):
                        t_, kk = src_at(tls, k)
                        nc.tensor.matmul(
                            pe[:, m, :],
                            lhsT=t_[:, kk, ts(m, P)],
                            rhs=t_[:, kk, :],
                            start=(k == 0),
                            stop=(k == KT - 1),
                        )

                # ---- softmax -> A''' = gamma*P/Z + I (natural layout) ----
                for m in range(2):
                    mu = sm_pool.tile([P, 1], f32, tag="mu")
                    Zs = sm_pool.tile([P, 1], f32, tag="Zs")
                    Zb = sm_pool.tile([P, 1], f16, tag="Zb")
                    rZ = sm_pool.tile([P, 1], f32, tag="rZ")
                    rZg = sm_pool.tile([P, 1], f32, tag="rZg")
                    Pm = sm_pool.tile([P, C], f16, tag="Pm")
                    nc.vector.tensor_reduce(
                        mu[:], pe[:, m, :], axis=mybir.AxisListType.X,
                        op=mybir.AluOpType.min,
                    )
                    nc.scalar.activation(
                        Pm[:],
                        pe[:, m, :],
                        mybir.ActivationFunctionType.Exp,
                        bias=mu[:],
                        scale=-1.0,
                        accum_out=Zs[:],
                    )
                    nc.vector.tensor_copy(Zb[:], Zs[:])
                    nc.vector.reciprocal(rZ[:], Zb[:])
                    nc.vector.tensor_scalar_mul(rZg[:], rZ[:], gv)
                    nc.scalar.mul(P2[:, m, :], Pm[:], rZg[:])
                    nc.gpsimd.tensor_add(
                        P2[:, m, ts(m, P)], P2[:, m, ts(m, P)], ident16[:]
                    )

                # At[j', 2m+jb, i'] = A'''[m*128 + i', jb*128 + j']
                # via 4 fp16 PE transposes; emitted AFTER matmul2 below for
                # b < NB-1 so they don't block the PE queue ahead of it.
                At = att_pool.tile([P, 4, P], f16, tag="At", name="At")

                def build_at(At=At, P2=P2):
                    ptl = psum_t.tile([P, 4, P], f16, name="ptl")
                    for m in range(2):
                        for jb in range(2):
                            nc.tensor.transpose(
                                ptl[:, 2 * m + jb, :],
                                P2[:, m, ts(jb, P)],
                                ident16[:],
                            )
                    for e in range(4):
                        nc.vector.tensor_copy(At[:, e, :], ptl[:, e, :])

                if b == NB - 1:
                    build_at()

                # ---- matmul2 for the PREVIOUS batch (software pipeline) ----
                this = (b, At, xn)
                todo = [pending] if pending is not None else []
                if b == NB - 1:
                    todo.append(this)
                    pending = None
                else:
                    pending = this
                for pb, pAt, pxn in todo:
                    for m in range(2):
                        ots = [
                            out_pool.tile(
                                [P, 2048], f16, tag=f"ot{c}", name=f"ot{c}"
                            )
                            for c in range(2)
                        ]
                        for g in range(TC // 4):
                            pos = [
                                psum_o.tile(
                                    [P, 1024], mybir.dt.float32,
                                    name="po", tag="po",
                                )
                                for j in range(2)
                            ]
                            for k in range(2):
                                for j in range(4):
                                    t8 = 4 * g + j
                                    nc.tensor.matmul(
                                        pos[j // 2][:, ts(j % 2, 512)],
                                        lhsT=pAt[:, 2 * m + k, :],
                                        rhs=pxn[:, k, ts(t8, 512)],
                                        start=(k == 0),
                                        stop=(k == 1),
                                    )
                            for jj in range(2):
                                nc.vector.tensor_copy(
                                    ots[g][:, ts(jj, 1024)], pos[jj][:]
                                )
                        for c in range(2):
                            nc.sync.dma_start(
                                o_ap[pb][:, m * T :][:, ts(c, 2048)], ots[c][:]
                            )

                if b < NB - 1:
                    build_at()

    nc.compile()
    return nc


def _get_nc():
    if "nc" not in _CACHE:
        _CACHE["nc"] = _build_nc()
    return _CACHE["nc"]


def _make_aux(gamma_val):
    aux = np.zeros((P, 132), dtype=np.float32)
    aux[:, 0] = gamma_val
    aux[:, 4:132] = np.eye(P, dtype=np.float32)
    return aux


def kernel(x, gamma, _trace=False):
    import concourse.bass_utils as bass_utils

    x = np.ascontiguousarray(np.asarray(x, dtype=np.float32))
    gamma = np.asarray(gamma, dtype=np.float32).reshape(-1)

    nc = _get_nc()

    aux = _make_aux(gamma[0])
    in_maps = []
    for d in range(NCORES):
        xs = x[d * NB : (d + 1) * NB]
        # xt[b, p, k*C+c] = x[b, c, k*128+p]  (fat contiguous runs)
        xt = np.ascontiguousarray(
            xs.transpose(0, 2, 1)
            .reshape(NB, KT, P, C)
            .transpose(0, 2, 1, 3)
            .reshape(NB, P, KT * C)
        )
        # xn[b, p, m*T+t] = x[b, m*128+p, t]
        xn = np.ascontiguousarray(
            xs.reshape(NB, 2, P, T).transpose(0, 2, 1, 3).reshape(NB, P, 2 * T)
        ).astype(np.float16)
        in_maps.append({"xt": xt, "xn": xn, "aux": aux})

    res = bass_utils.run_bass_kernel_spmd(
        nc, in_maps, core_ids=list(range(NCORES)), trace=_trace
    )
    # o[b, p, m*T + t] = out[b, m*128+p, t]
    out = np.concatenate(
        [
            r["o"].reshape(NB, P, 2, T).transpose(0, 2, 1, 3).reshape(NB, C, T)
            for r in res.results
        ],
        axis=0,
    ).astype(np.float32)
    if _trace:
        _CACHE["last_results"] = res
    return out


# revision 28
# speedup vs baseline: 1.3720x; 1.3720x over previous
"""Trainium2 Bass kernel for nn_Attention_Module (sparse_attention).

Computation per batch b (x_b: [C=256, T=4096] fp32):
    energy = x_b @ x_b^T                      # (256, 256), K=4096
    attn   = softmax(rowmax(energy) - energy) # == exp(mu - e)/Z, mu = rowmin
    out    = gamma * (attn @ x_b) + x_b

Strategy (8 cores, pure data-parallel, 4 batches/core):
  - Host pre-swizzles both x layouts so every DMA transfer is a fat
    contiguous run (16-32 KB per partition): 1 KB-granular descriptors
    previously made HWDGE dispatch the bottleneck (~10 us per 2 MB load).
  - xt (t-on-partition, fp32) feeds the energy matmul as float32r;
    xn (c-on-partition, fp16) feeds the second matmul and the residual.
  - The +x residual is folded into matmul2 via the modified attention matrix
    A'' = gamma*P^T + diag(Z); out = diag(1/Z) * (A''^T @ x).
  - Output is stored fp16 (tolerance is 2e-2); host upcasts to fp32.
"""

import numpy as np

B, C, T = 32, 256, 4096
NCORES = 8
NB = B // NCORES  # batches per core
P = 128
KT = T // P  # 32 k-tiles for the energy matmul
TC = T // 512  # 8 t-chunks for matmul2

_CACHE = {}


def _build_nc(variant=None):
    variant = variant or {}
    from contextlib import ExitStack

    import concourse.bacc as bacc
    import concourse.bass as bass
    import concourse.tile as tile
    from concourse import mybir

    f32 = mybir.dt.float32
    f32r = mybir.dt.float32r
    f16 = mybir.dt.float16
    ts = bass.ts

    nc = bacc.Bacc(
        "TRN2",
        target_bir_lowering=False,
        debug=False,
        enable_asserts=False,
        num_devices=NCORES,
    )

    # host-swizzled layouts: contiguous fat runs per partition
    xt_h = nc.dram_tensor("xt", [NB, P, KT * C], f32r, kind="ExternalInput")
    xn_h = nc.dram_tensor("xn", [NB, P, 2 * T], f16, kind="ExternalInput")
    # aux: per-partition row [gamma, pad, pad, pad, identity-row(128)]
    aux_h = nc.dram_tensor("aux", [P, 132], f32, kind="ExternalInput")
    o_h = nc.dram_tensor("o", [NB, P, 2 * T], f16, kind="ExternalOutput")

    with tile.TileContext(nc) as tc:
        with ExitStack() as ctx:
            singles = ctx.enter_context(tc.tile_pool(name="singles", bufs=1))
            xt_pool = ctx.enter_context(tc.tile_pool(name="xt", bufs=2))
            xq_pool = ctx.enter_context(tc.tile_pool(name="xq", bufs=1))
            xn_pool = ctx.enter_context(tc.tile_pool(name="xn", bufs=3))
            out_pool = ctx.enter_context(tc.tile_pool(name="out", bufs=2))
            att_pool = ctx.enter_context(tc.tile_pool(name="att", bufs=3))
            small = ctx.enter_context(tc.tile_pool(name="small", bufs=4))
            psum_e = ctx.enter_context(
                tc.tile_pool(name="psum_e", bufs=2, space="PSUM")
            )
            psum_t = ctx.enter_context(
                tc.tile_pool(name="psum_t", bufs=2, space="PSUM")
            )
            psum_o = ctx.enter_context(
                tc.tile_pool(name="psum_o", bufs=4, space="PSUM")
            )

            xt_ap = xt_h.ap()
            xn_ap = xn_h.ap()
            o_ap = o_h.ap()

            # aux on the ACT ring so it doesn't delay the first xt load
            aux = singles.tile([P, 132], f32)
            nc.scalar.dma_start(aux[:], aux_h.ap())
            gv = aux[:, 0:1]
            ident = aux[:, 4:132]

            KH = KT // 2

            def issue_loads(b):
                if b == 0:
                    # four separate tiles so matmul1 starts after the first
                    # 1 MB lands (Tile tracks deps at tile granularity)
                    KQ = KT // 4
                    qs = []
                    for q in range(4):
                        t_ = xq_pool.tile(
                            [P, KQ, C], f32r, tag=f"xq{q}", name=f"xq{q}"
                        )
                        nc.sync.dma_start(
                            t_[:],
                            xt_ap[b][:, q * KQ * C : (q + 1) * KQ * C],
                        )
                        qs.append(t_)
                    xt_tiles, kdiv = qs, KQ
                else:
                    xta = xt_pool.tile([P, KH, C], f32r, tag="xta", name="xta")
                    xtb = xt_pool.tile([P, KH, C], f32r, tag="xtb", name="xtb")
                    nc.sync.dma_start(xta[:], xt_ap[b][:, : KH * C])
                    nc.sync.dma_start(xtb[:], xt_ap[b][:, KH * C :])
                    xt_tiles, kdiv = [xta, xtb], KH
                xn = xn_pool.tile([P, 2, T], f16, tag="xn", name="xn")
                nc.sync.dma_start(xn[:], xn_ap[b])
                return xt_tiles, kdiv, xn

            tiles = {0: issue_loads(0)}
            pending = None  # (b, At, rZ, xn) awaiting matmul2

            for b in range(NB):
                xt, kdiv, xn = tiles.pop(b)
                if b + 1 < NB:
                    tiles[b + 1] = issue_loads(b + 1)

                # A''^T, laid out [128(j within k-block), k-block, 256(i)]
                At = att_pool.tile([P, 2, C], f16)
                Zs = small.tile([P, 2], f32)
                Zb = small.tile([P, 2], f16)
                rZ = small.tile([P, 2], f32)

                for m in range(2):
                    pe = psum_e.tile([P, C], mybir.dt.float32)
                    for k in range(KT):
                        src_t = xt[k // kdiv]
                        kk = k % kdiv
                        nc.tensor.matmul(
                            pe[:],
                            lhsT=src_t[:, kk, ts(m, P)],
                            rhs=src_t[:, kk, :],
                            start=(k == 0),
                            stop=(k == KT - 1),
                        )
                    mu = small.tile([P, 1], f32)
                    nc.vector.tensor_reduce(
                        mu[:], pe[:], axis=mybir.AxisListType.X,
                        op=mybir.AluOpType.min,
                    )
                    Pm = small.tile([P, C], f32, tag="Pm")
                    nc.scalar.activation(
                        Pm[:],
                        pe[:],
                        mybir.ActivationFunctionType.Exp,
                        bias=mu[:],
                        scale=-1.0,
                        accum_out=Zs[:, m : m + 1],
                    )
                    nc.vector.tensor_copy(Zb[:, m : m + 1], Zs[:, m : m + 1])
                    nc.vector.reciprocal(rZ[:, m : m + 1], Zb[:, m : m + 1])
                    for k in range(2):
                        pt = psum_t.tile([P, P], mybir.dt.float32)
                        nc.tensor.transpose(pt[:], Pm[:, ts(k, P)], ident)
                        # A''T[j in k-block, i in m-block] = gamma * P^T
                        nc.scalar.mul(At[:, k, ts(m, P)], pt[:], gv)
                    # diagonal: += diag(Z) (falls in the k == m block)
                    dg = small.tile([P, P], f16, tag="diag")
                    nc.vector.tensor_scalar_mul(dg[:], ident, Zs[:, m : m + 1])
                    nc.vector.tensor_add(
                        At[:, m, ts(m, P)], At[:, m, ts(m, P)], dg[:]
                    )

                # software-pipeline the PE: run the PREVIOUS batch's matmul2
                # after this batch's matmul1, hiding the A'' build latency.
                this = (b, At, rZ, xn)
                todo = [pending] if pending is not None else []
                if b == NB - 1:
                    todo.append(this)
                    pending = None
                else:
                    pending = this
                for pb, pAt, prZ, pxn in todo:
                    for m in range(2):
                        ot = out_pool.tile([P, T], f16, tag="ot", name="ot")
                        for t8 in range(TC):
                            po = psum_o.tile([P, 512], mybir.dt.float32)
                            for k in range(2):
                                nc.tensor.matmul(
                                    po[:],
                                    lhsT=pAt[:, k, ts(m, P)],
                                    rhs=pxn[:, k, ts(t8, 512)],
                                    start=(k == 0),
                                    stop=(k == 1),
                                )
                            # out = psum * (1/Z); alternate engines
                            if t8 % 2 == 0:
                                nc.vector.tensor_scalar_mul(
                                    ot[:, ts(t8, 512)], po[:], prZ[:, m : m + 1]
                                )
                            else:
                                nc.scalar.mul(
                                    ot[:, ts(t8, 512)], po[:], prZ[:, m : m + 1]
                                )
                        nsplit = 4 if pb == NB - 1 else 2
                        for sh in range(nsplit):
                            nc.sync.dma_start(
                                o_ap[pb][:, m * T :][:, ts(sh, T // nsplit)],
                                ot[:, ts(sh, T // nsplit)],
                            )

    nc.compile()
    return nc


def _get_nc():
    if "nc" not in _CACHE:
        _CACHE["nc"] = _build_nc()
    return _CACHE["nc"]


def _make_aux(gamma_val):
    aux = np.zeros((P, 132), dtype=np.float32)
    aux[:, 0] = gamma_val
    aux[:, 4:132] = np.eye(P, dtype=np.float32)
    return aux


def kernel(x, gamma, _trace=False):
    import concourse.bass_utils as bass_utils

    x = np.ascontiguousarray(np.asarray(x, dtype=np.float32))
    gamma = np.asarray(gamma, dtype=np.float32).reshape(-1)

    nc = _get_nc()

    aux = _make_aux(gamma[0])
    in_maps = []
    for d in range(NCORES):
        xs = x[d * NB : (d + 1) * NB]
        # xt[b, p, k*C+c] = x[b, c, k*128+p]  (fat contiguous runs)
        xt = np.ascontiguousarray(
            xs.transpose(0, 2, 1)
            .reshape(NB, KT, P, C)
            .transpose(0, 2, 1, 3)
            .reshape(NB, P, KT * C)
        )
        # xn[b, p, m*T+t] = x[b, m*128+p, t]
        xn = np.ascontiguousarray(
            xs.reshape(NB, 2, P, T).transpose(0, 2, 1, 3).reshape(NB, P, 2 * T)
        ).astype(np.float16)
        in_maps.append({"xt": xt, "xn": xn, "aux": aux})

    res = bass_utils.run_bass_kernel_spmd(
        nc, in_maps, core_ids=list(range(NCORES)), trace=_trace
    )
    # o[b, p, m*T+t] = out[b, m*128+p, t]
    out = np.concatenate(
        [
            r["o"].reshape(NB, P, 2, T).transpose(0, 2, 1, 3).reshape(NB, C, T)
            for r in res.results
        ],
        axis=0,
    ).astype(np.float32)
    if _trace:
        _CACHE["last_results"] = res
    return out
